# revision 23
# baseline (speedup 1.0000x reference)
"""Multi-head attention (B=4, S=2048, D=1024, H=16, causal) on 8 trn2 cores.

Sharding: core c -> (batch b = c//2, head-half g = c%2, heads g*8..g*8+8).
Each core computes QKV projections for its 8 heads, causal attention, and a
partial dense projection (its 512 input dims). Host sums core pairs + bias.

On-chip layout (per core), all matmuls bf16 with fp32 PSUM accumulate:
  q/k/v     host-prepacked [(chunk,t,p), m] so chunk DMAs are contiguous
  QT/KT     [m, s]   m = 8*64 head dims on partitions (4 chunks of 128),
                     split per 1024-col s-chunk; Q/K bias folded into the
                     PSUM eviction via per-partition tensor_scalar_add
  VA        [s, 8*65] per 128-row s-tile; col 64 of each 65-group = ones
                     (gives softmax denominators via the P@V matmul)
  attention: ST = K_h @ Q_h^T -> [s_k part, s_q free] (two heads row-packed
             via tile_position); causal mask folded into the ST accumulation
             as one extra matmul (maskA^T @ eye adds -1e9 above the
             diagonal); exp on ACT (scale=1/8, no max-subtraction)
  O psum    [65, s_q]: rows 0:64 = unnormalized O^T, row 64 = softmax sums;
             normalize = reciprocal_approx_fast + gpsimd partition_broadcast
             + DVE mul (no DRAM round trip)
  dense     out[s, 1024] partial = O^T.T @ dwT, evicted fp32
"""

import sys

sys.path.insert(0, "/opt/trn_rl_repo")

import numpy as np
import ml_dtypes

import concourse.bass as bass
import concourse.mybir as mybir
import concourse.tile as tile
from concourse.bass_utils import run_bass_kernel_spmd

BF16 = mybir.dt.bfloat16
F32 = mybir.dt.float32
bf16 = ml_dtypes.bfloat16

B, S, D, H, DEPTH = 4, 2048, 1024, 16, 64
NCORES = 8
HPC = H // 2  # 8 heads per core
M = HPC * DEPTH  # 512 head dims per core
CH = 1024  # attention s_q chunk width
NCH = S // CH  # 2
NKT = S // 128  # 16 k tiles
NEG = -1.0e9
EXPF = mybir.ActivationFunctionType.Exp

_CACHE = {}


def _x_dma(tc, sc, which, xch, tensors):
    """DMA one input tensor chunk (contiguous 8KB/partition lines)."""
    nc = tc.nc
    (qr, kr, vr) = tensors[0:3]
    src = {"q": qr, "k": kr, "v": vr}[which]
    t = xch.tile([128, 8, 512], BF16, tag=f"{which}_ch", name=f"{which}_ch{sc}")
    nc.sync.dma_start(out=t, in_=src[:, sc])
    return t


def _qk_mt(tc, pr, mt, chs, pjps, tensors):
    """Q+K projections for head-pair mt over chunk pair pr (both 512-col
    halves), so attention unit (pr, mt) becomes runnable immediately."""
    nc = tc.nc
    (qr, kr, vr, wq_sb, wk_sb, wv_sb, bq_sb, bk_sb, bvb_sb, QT, KT, VA) = tensors
    msl = bass.ts(mt, 128)
    for w_sb, b_sb, DST in ((wq_sb, bq_sb, QT), (wk_sb, bk_sb, KT)):
        for sc in (2 * pr, 2 * pr + 1):
            x_ch = chs[sc][0 if DST is QT else 1]
            csl = bass.ts(sc % 2, 512)
            ps = pjps.tile([128, 512], F32, tag="ST", name=f"pj{pr}_{mt}_{sc}_{0 if DST is QT else 1}")
            for t in range(8):
                nc.tensor.matmul(
                    ps, w_sb[:, t, msl], x_ch[:, t, :], start=(t == 0), stop=(t == 7)
                )
            nc.vector.tensor_scalar_add(DST[(mt, pr)][:, csl], ps, b_sb[:, mt : mt + 1])


def _v_chunk(tc, sc, chs, pjps, tensors):
    """V projection for one 512-row s-chunk -> VA tiles."""
    nc = tc.nc
    (qr, kr, vr, wq_sb, wk_sb, wv_sb, bq_sb, bk_sb, bvb_sb, QT, KT, VA) = tensors
    v_ch = chs[sc][2]
    for sti in range(4):  # V: [s part, m free]
        st = sc * 4 + sti
        ps_v = pjps.tile([128, 512], F32, tag="ST", name=f"psv{st}")
        for t in range(8):
            nc.tensor.matmul(
                ps_v, v_ch[:, t, bass.ts(sti, 128)], wv_sb[:, t, :],
                start=(t == 0), stop=(t == 7),
            )
        nc.vector.tensor_add(
            VA[st].rearrange("p (h c) -> p h c", c=65)[:, :, 0:64],
            ps_v.rearrange("p (h c) -> p h c", c=64),
            bvb_sb.rearrange("p (h c) -> p h c", c=64),
        )


def _attn_unit(tc, c, hp, pools, tensors):
    """Causal attention for s_q chunk c, head pair hp (heads 2hp, 2hp+1)."""
    nc = tc.nc
    stps, ops, epool, rpool, bcpool, tpool = pools
    (QT, KT, VA, OT, maskA_sb, den_all, rb) = tensors
    O1 = ops.tile([128, CH], F32, tag="O1", name=f"O1_{c}_{hp}")
    O2 = ops.tile([128, CH], F32, tag="O2", name=f"O2_{c}_{hp}")
    nkt = 8 * (c + 1)
    last0 = min(nkt - 1, 8 * c + 3)  # last kt writing cols [0:512)
    for kt in range(nkt):
        j = kt - 8 * c
        qoff = 128 * j if j > 0 else 0
        diag_hf = qoff // 512 if j >= 0 else -1
        ST1 = stps.tile([128, CH], F32, tag="ST", name=f"ST1_{c}_{hp}_{kt}")
        ST2 = stps.tile([128, CH], F32, tag="ST", name=f"ST2_{c}_{hp}_{kt}")
        for hf in (0, 1):
            lo, hi = 512 * hf, 512 * (hf + 1)
            if qoff >= hi:
                continue
            off = max(qoff, lo)
            for idx in (0, 1):
                STx = ST1 if idx == 0 else ST2
                bp = idx * 64
                nc.tensor.matmul(
                    STx[:, off:hi],
                    KT[(hp, kt // 8)][bp : bp + 64, bass.ts(kt % 8, 128)],
                    QT[(hp, c)][bp : bp + 64, off:hi],
                    start=True, stop=True,
                    tile_position=(bp, 0),
                )
        E1 = epool.tile([128, CH], BF16, tag="E1", name=f"E1_{c}_{hp}_{kt}")
        E2 = epool.tile([128, CH], BF16, tag="E2", name=f"E2_{c}_{hp}_{kt}")
        nc.scalar.activation(E1[:, qoff:CH], ST1[:, qoff:CH], EXPF, scale=0.125)
        nc.scalar.activation(E2[:, qoff:CH], ST2[:, qoff:CH], EXPF, scale=0.125)
        if diag_hf >= 0:
            # causal mask: zero E strictly below the diagonal of the
            # 128x128 diagonal block (multiplicative triu mask), on the
            # otherwise-idle gpsimd engine to keep PE/ACT free
            dsl = bass.ds(qoff, 128)
            nc.gpsimd.tensor_mul(E1[:, dsl], E1[:, dsl], maskA_sb)
            nc.gpsimd.tensor_mul(E2[:, dsl], E2[:, dsl], maskA_sb)
        for hf in (0, 1):
            lo, hi = 512 * hf, 512 * (hf + 1)
            if qoff >= hi:
                continue
            off = max(qoff, lo)
            lastk = last0 if hf == 0 else nkt - 1
            for idx, Ox, Ex in ((0, O1, E1), (1, O2, E2)):
                h = 2 * hp + idx
                nc.tensor.matmul(
                    Ox[0:65, off:hi],
                    VA[kt][:, h * 65 : (h + 1) * 65],
                    Ex[:, off:hi],
                    start=(kt == 0), stop=(kt == lastk),
                )
    # evict UNNORMALIZED (frees the O psum slot fast); scatter the softmax
    # sums (row 64) into den_t transposed [8 rows -> 8 partitions each] so
    # the batched per-chunk reciprocal runs 128 elems/lane instead of 1024
    den_t = tensors[-2]
    for idx, Ox in ((0, O1), (1, O2)):
        s = hp * 2 + idx
        dr = rpool.tile([1, CH], F32, tag="dr", name=f"dr{c}_{s}")
        nc.vector.tensor_copy(dr, Ox[64:65, :])
        nc.gpsimd.dma_start(out=den_t[8 * s : 8 * s + 8, :], in_=dr)
        if idx == 0:
            nc.vector.tensor_copy(OT[(hp, c)][0:64, :], Ox[0:64, :])
        else:
            tmp = tpool.tile([64, CH], BF16, tag="tmp", name=f"tmp{c}_{s}")
            nc.vector.tensor_copy(tmp, Ox[0:64, :])
            nc.gpsimd.dma_start(out=OT[(hp, c)][64:128, :], in_=tmp)


def _attn_normalize(tc, c, pools, tensors):
    """Batched softmax normalization for all 4 head-pairs of chunk c:
    one [64, 128] reciprocal, DRAM stride-0 broadcast, in-place DVE mul."""
    nc = tc.nc
    stps, ops, epool, rpool, bcpool, tpool = pools
    (QT, KT, VA, OT, maskA_sb, den_t, rb) = tensors
    nc.vector.reciprocal(den_t, den_t)
    rbf = rpool.tile([64, 128], BF16, tag="rbf", name=f"rbf{c}")
    nc.vector.tensor_copy(rbf, den_t)
    nc.gpsimd.dma_start(
        out=rb[:, :].rearrange("s (j m) -> (s j) m", m=128), in_=rbf
    )
    for hp in range(4):
        bc = bcpool.tile([128, CH], BF16, tag="bc", name=f"bc{c}_{hp}")
        for idx in (0, 1):
            src = rb[hp * 2 + idx : hp * 2 + idx + 1, :]
            nc.gpsimd.dma_start(
                out=bc[idx * 64 : (idx + 1) * 64, :],
                in_=bass.AP(tensor=src.tensor, offset=src.offset, ap=[[0, 64], [1, CH]]),
            )
        nc.vector.tensor_mul(OT[(hp, c)], OT[(hp, c)], bc)


def _dense_st(tc, st, dnps, osb, dw_sb, OT, out):
    nc = tc.nc
    c = st // 8
    ssl = bass.ds((st % 8) * 128, 128)
    for nh in range(2):
        ps = dnps.tile([128, 512], F32, tag="ST", name=f"dn{st}_{nh}")
        for mt in range(4):
            nc.tensor.matmul(
                ps, OT[(mt, c)][:, ssl], dw_sb[:, mt, bass.ts(nh, 512)],
                start=(mt == 0), stop=(mt == 3),
            )
        o_sb = osb.tile([128, 512], F32, tag="o_sb", name=f"o_sb{st}_{nh}")
        if st < 8:
            nc.vector.tensor_copy(o_sb, ps)
        else:
            nc.scalar.copy(o_sb, ps)
        nc.sync.dma_start(out=out[bass.ts(st, 128), bass.ts(nh, 512)], in_=o_sb)


def _body(tc):
    nc = tc.nc
    dram = {t.name: t for t in _CACHE["dram"]}
    out = dram["out"]

    # ---- persistent tiles (one bufs=1 pool, distinct tags -> own slots) ----
    import contextlib
    _pc = contextlib.ExitStack()
    persist = _pc.enter_context(tc.tile_pool(name="persist", bufs=1))

    def P(shape, dt, name):
        return persist.tile(shape, dt, tag=name, name=name)

    wq_sb = P([128, 8, M], BF16, "wq_sb")
    wk_sb = P([128, 8, M], BF16, "wk_sb")
    wv_sb = P([128, 8, M], BF16, "wv_sb")
    dw_sb = P([128, 4, D], BF16, "dw_sb")
    bq_sb = P([128, 4], F32, "bq_sb")
    bk_sb = P([128, 4], F32, "bk_sb")
    bvb_sb = P([128, M], F32, "bvb_sb")
    maskA_sb = P([128, 128], BF16, "maskA_sb")

    QT = {
        (mt, c): P([128, CH], BF16, f"QT{mt}_{c}")
        for mt in range(4)
        for c in range(NCH)
    }
    KT = {
        (mt, c): P([128, CH], BF16, f"KT{mt}_{c}")
        for mt in range(4)
        for c in range(NCH)
    }
    VA = {st: P([128, HPC * 65], BF16, f"VA{st}") for st in range(NKT)}
    OT = {
        (hp, c): P([128, CH], BF16, f"OT{hp}_{c}")
        for hp in range(4)
        for c in range(NCH)
    }
    for st in range(NKT):
        nc.gpsimd.memset(VA[st], 1.0)

    qr = dram["qpk"][:, :, :, :]
    kr = dram["kpk"][:, :, :, :]
    vr = dram["vpk"][:, :, :, :]
    rb = _CACHE["rb"]
    ptens = (qr, kr, vr, wq_sb, wk_sb, wv_sb, bq_sb, bk_sb, bvb_sb, QT, KT, VA)

    with (
        tc.tile_pool(name="xch", bufs=2) as xch,
        tc.tile_pool(name="stps", bufs=2, space="PSUM") as stps,
        tc.tile_pool(name="ops", bufs=1, space="PSUM") as ops,
        tc.tile_pool(name="epool", bufs=3) as epool,
        tc.tile_pool(name="rpool", bufs=2) as rpool,
        tc.tile_pool(name="bcpool", bufs=2) as bcpool,
        tc.tile_pool(name="tpool", bufs=2) as tpool,
        tc.tile_pool(name="osb", bufs=2) as osb,
        tc.tile_pool(name="dpool", bufs=2) as dpool,
    ):
        apools = (stps, ops, epool, rpool, bcpool, tpool)
        # Emission order = Tile priority. DMAs are emitted in consumption
        # order; Q/K projections are interleaved per head-pair (mt) across
        # each chunk pair so attention unit (0, mt) becomes runnable as
        # soon as its own Q/K tiles land (~15us in), keeping ACT busy from
        # the start. Chunk-pair-1 projections are emitted after attention
        # c0 as PE filler for its ACT-bound stretches, and spill into the
        # start of attention c1 (whose kt 0..7 only need K/V chunks 0-1).
        chs = {}
        nc.sync.dma_start(out=wq_sb, in_=dram["wqpk"][:, :, :])
        nc.sync.dma_start(out=bq_sb, in_=dram["bqc"][:, :])
        chs[0] = [_x_dma(tc, 0, "q", xch, ptens), None, None]
        chs[1] = [_x_dma(tc, 1, "q", xch, ptens), None, None]
        nc.sync.dma_start(out=wk_sb, in_=dram["wkpk"][:, :, :])
        nc.sync.dma_start(out=bk_sb, in_=dram["bkc"][:, :])
        chs[0][1] = _x_dma(tc, 0, "k", xch, ptens)
        chs[1][1] = _x_dma(tc, 1, "k", xch, ptens)
        nc.sync.dma_start(out=wv_sb, in_=dram["wvpk"][:, :, :])
        nc.sync.dma_start(out=bvb_sb, in_=dram["bvb"][:, :])
        nc.sync.dma_start(out=maskA_sb, in_=dram["maskA"][:, :])
        chs[0][2] = _x_dma(tc, 0, "v", xch, ptens)
        chs[1][2] = _x_dma(tc, 1, "v", xch, ptens)

        den0 = dpool.tile([64, 128], F32, tag="den", name="den0")
        atens0 = (QT, KT, VA, OT, maskA_sb, den0, rb)
        _qk_mt(tc, 0, 0, chs, stps, ptens)
        _v_chunk(tc, 0, chs, stps, ptens)
        _v_chunk(tc, 1, chs, stps, ptens)
        _attn_unit(tc, 0, 0, apools, atens0)
        _qk_mt(tc, 0, 1, chs, stps, ptens)
        _attn_unit(tc, 0, 1, apools, atens0)
        _qk_mt(tc, 0, 2, chs, stps, ptens)
        _attn_unit(tc, 0, 2, apools, atens0)
        _qk_mt(tc, 0, 3, chs, stps, ptens)
        _attn_unit(tc, 0, 3, apools, atens0)

        # chunk pair 1 projections: PE filler for attn c0 + early attn c1
        chs[2] = [_x_dma(tc, 2, "q", xch, ptens), None, None]
        chs[3] = [_x_dma(tc, 3, "q", xch, ptens), None, None]
        chs[2][1] = _x_dma(tc, 2, "k", xch, ptens)
        chs[3][1] = _x_dma(tc, 3, "k", xch, ptens)
        chs[2][2] = _x_dma(tc, 2, "v", xch, ptens)
        chs[3][2] = _x_dma(tc, 3, "v", xch, ptens)
        nc.sync.dma_start(out=dw_sb, in_=dram["dwpk"][:, :, :])
        for mt in range(4):
            _qk_mt(tc, 1, mt, chs, stps, ptens)
        _v_chunk(tc, 2, chs, stps, ptens)
        _v_chunk(tc, 3, chs, stps, ptens)
        _attn_normalize(tc, 0, apools, atens0)

        den1 = dpool.tile([64, 128], F32, tag="den", name="den1")
        atens1 = (QT, KT, VA, OT, maskA_sb, den1, rb)
        for hp in range(4):
            _attn_unit(tc, 1, hp, apools, atens1)
        _attn_normalize(tc, 1, apools, atens1)
        for st in range(NKT):
            _dense_st(tc, st, stps, osb, dw_sb, OT, out)
    _pc.close()


def _legalize_dma_waits(nc):
    """Walrus accepts only one sync wait per instruction (EventSemaphore: 2,
    Drain: special-cased). Spill extra waits onto preceding InstEventSemaphore
    ops on the same engine sequencer."""
    for f in nc.m.functions:
        for blk in f.blocks:
            new_insts = []
            for inst in blk.instructions:
                si = getattr(inst, "sync_info", None)
                exempt = isinstance(inst, mybir.InstEventSemaphore)
                if not exempt and si is not None and len(si.on_wait) > 1:
                    waits = list(si.on_wait)
                    extra, keep = waits[:-1], waits[-1:]
                    while extra:
                        chunk, extra = extra[:2], extra[2:]
                        new_insts.append(
                            mybir.InstEventSemaphore(
                                name=nc.get_next_instruction_name(),
                                engine=inst.engine,
                                ins=[],
                                outs=[],
                                sync_info=mybir.SyncInfo(on_wait=chunk, on_update=[]),
                            )
                        )
                    inst.sync_info = mybir.SyncInfo(
                        on_wait=keep, on_update=list(si.on_update)
                    )
                new_insts.append(inst)
            blk.instructions[:] = new_insts


def _build():
    nc = bass.Bass()
    dram = [
        nc.declare_dram_parameter("qpk", [128, 4, 8, 512], BF16, isOutput=False),
        nc.declare_dram_parameter("kpk", [128, 4, 8, 512], BF16, isOutput=False),
        nc.declare_dram_parameter("vpk", [128, 4, 8, 512], BF16, isOutput=False),
        nc.declare_dram_parameter("wqpk", [128, 8, M], BF16, isOutput=False),
        nc.declare_dram_parameter("wkpk", [128, 8, M], BF16, isOutput=False),
        nc.declare_dram_parameter("wvpk", [128, 8, M], BF16, isOutput=False),
        nc.declare_dram_parameter("dwpk", [128, 4, D], BF16, isOutput=False),
        nc.declare_dram_parameter("bqc", [128, 4], F32, isOutput=False),
        nc.declare_dram_parameter("bkc", [128, 4], F32, isOutput=False),
        nc.declare_dram_parameter("bvb", [128, M], F32, isOutput=False),
        nc.declare_dram_parameter("maskA", [128, 128], BF16, isOutput=False),
        nc.declare_dram_parameter("out", [S, D], F32, isOutput=True),
    ]
    _CACHE["dram"] = dram
    _CACHE["rb"] = nc.dram_tensor("rb", [8, CH], BF16)
    with tile.TileContext(nc) as tc:
        _body(tc)
    _legalize_dma_waits(nc)
    return nc


def _get_nc():
    if "nc" not in _CACHE:
        _CACHE["nc"] = _build()
    return _CACHE["nc"]


def _pack_x(xT):
    # [D, S] -> [p, chunk, t, 512]: per (partition, chunk) an 8KB
    # contiguous run, so chunk DMAs use 128 fat descriptors
    return np.ascontiguousarray(xT.reshape(8, 128, 4, 512).transpose(1, 2, 0, 3))


def _make_in_maps(q, k, v, wq_w, wq_b, wk_w, wk_b, wv_w, wv_b, dense_w, dense_b):
    q, k, v = (np.asarray(x, np.float32) for x in (q, k, v))
    maskA = np.triu(np.ones((128, 128), np.float32)).astype(bf16)
    in_maps = []
    for core in range(NCORES):
        b, g = divmod(core, 2)
        hs = slice(g * M, (g + 1) * M)
        wqT = np.asarray(wq_w)[hs].T.astype(bf16)  # [D, M]
        wkT = np.asarray(wk_w)[hs].T.astype(bf16)
        wvT = np.asarray(wv_w)[hs].T.astype(bf16)
        dwT = np.asarray(dense_w)[:, hs].T.astype(bf16)  # [M, D]
        in_maps.append(
            {
                "qpk": _pack_x(q[b].T.astype(bf16)),
                "kpk": _pack_x(k[b].T.astype(bf16)),
                "vpk": _pack_x(v[b].T.astype(bf16)),
                "wqpk": np.ascontiguousarray(
                    wqT.reshape(8, 128, M).transpose(1, 0, 2)
                ),
                "wkpk": np.ascontiguousarray(
                    wkT.reshape(8, 128, M).transpose(1, 0, 2)
                ),
                "wvpk": np.ascontiguousarray(
                    wvT.reshape(8, 128, M).transpose(1, 0, 2)
                ),
                "dwpk": np.ascontiguousarray(
                    dwT.reshape(4, 128, D).transpose(1, 0, 2)
                ),
                "bqc": np.ascontiguousarray(
                    np.asarray(wq_b, np.float32)[hs].reshape(4, 128).T
                ),
                "bkc": np.ascontiguousarray(
                    np.asarray(wk_b, np.float32)[hs].reshape(4, 128).T
                ),
                "bvb": np.ascontiguousarray(
                    np.broadcast_to(np.asarray(wv_b, np.float32)[hs], (128, M))
                ),
                "maskA": maskA,
            }
        )
    return in_maps


def kernel(q, k, v, wq_w, wq_b, wk_w, wk_b, wv_w, wv_b, dense_w, dense_b):
    nc = _get_nc()
    in_maps = _make_in_maps(
        q, k, v, wq_w, wq_b, wk_w, wk_b, wv_w, wv_b, dense_w, dense_b
    )
    res = run_bass_kernel_spmd(nc, in_maps, list(range(NCORES)))
    _CACHE["last_res"] = res
    outs = [r["out"] for r in res.results]
    final = np.empty((B, S, D), np.float32)
    db = np.asarray(dense_b, np.float32)
    for b in range(B):
        final[b] = outs[2 * b] + outs[2 * b + 1] + db[None, :]
    return final
